# revision 9
# baseline (speedup 1.0000x reference)
"""Multi-head causal attention (B=2, S=2048, D=1024, H=16) on 8 trn2 cores.

Sharding: core c handles batch b = c // 4 and head group g = c % 4 (4 heads,
256 feature columns). Each core computes its heads' attention context and a
partial output projection (ctx_g @ Wo[rows_g]); the host sums the 4 partials
per batch and adds bo.

v3 design (all matmul operands bf16, fp32 PSUM/denominators/output):
- K^T and Q^T share the layout [128, hm, S]: head 2*hm in partitions 0:63,
  head 2*hm+1 in 64:127.  Score matmuls contract over K=64 at base partition
  0 / 64; bass auto-derives tile_position from base_partition, so the two
  heads' score matmuls are emitted adjacently and row-pack into disjoint
  halves of the PE array (concurrent execution, ~2x score throughput).
- Normalization without the DRAM-bounce broadcast: denominator row ->
  reciprocal (DVE) -> PE outer product ones[1,64] x recip[1,512] into PSUM
  -> tensor_tensor multiply.  Latency ~2us instead of ~9us of DMA hops.
- The whole kernel is emitted as fine-grained units: attention (n, hm, ski)
  steps are interleaved 1:k with projection / output-projection chunks so
  the exp (ACT engine) always has PE work to hide behind and the PE never
  idles >3.4us (HAM stays at full clock).
- All PSUM drains on DVE; ACT runs only the exps (same act-table set).
"""

import os
import sys
import types
from contextlib import ExitStack

import numpy as np
import ml_dtypes

import concourse.bacc as bacc
import concourse.bass as bass
import concourse.mybir as mybir
import concourse.tile as tile
from concourse.bass_utils import run_bass_kernel_spmd


def _install_ntff_hook():
    """The agent image's antenv lacks axon_hooks, so trn_boot's NTFF hook
    install degrades silently. Recreate the module + hook so trace=True works."""
    if "antenv.axon_hooks" in sys.modules:
        return
    try:
        mod = types.ModuleType("antenv.axon_hooks")
        holder = [None]
        mod.set_axon_ntff_profile_hook = lambda h: holder.__setitem__(0, h)
        mod.get_axon_ntff_profile_hook = lambda: holder[0]
        from trn_agent_boot.trn_boot import _ntff_profile_via_ctypes

        hook = _ntff_profile_via_ctypes("/opt/axon/libaxon_pjrt.so")
        if hook is None:
            return
        mod.set_axon_ntff_profile_hook(hook)
        sys.modules["antenv.axon_hooks"] = mod
    except Exception:
        pass

B, S, D, H, HD = 2, 2048, 1024, 16, 64
NCORES = 8
GROUPS = 4          # head groups (cores) per batch
HC = H // GROUPS    # heads per core
DG = HC * HD        # feature columns per core (256)
P = 128
KSUB = D // P       # 8 contraction subtiles for the projections
SQT = 512           # sq tile width (free dim of scores/ctx matmuls)
NSQ = S // SQT      # 4
NST = S // P        # 16 s subtiles of 128
F32 = mybir.dt.float32
F32R = mybir.dt.float32r
BF = mybir.dt.bfloat16
EXP = mybir.ActivationFunctionType.Exp

_CACHE = {}


def _mha_tile_kernel(tc, xT, wq, wk, wv, wo, out):
    nc = tc.nc
    scale = 1.0 / np.sqrt(np.float32(HD))

    with ExitStack() as ctx:
        consts = ctx.enter_context(tc.tile_pool(name="consts", bufs=1))
        # PSUM budget: pps 1-bank x2 + sps 2-bank x2 + cps 1-bank x2 = 8 banks
        pps = ctx.enter_context(tc.tile_pool(name="pps", bufs=2, space="PSUM"))
        sps = ctx.enter_context(tc.tile_pool(name="sps", bufs=2, space="PSUM"))
        cps = ctx.enter_context(tc.tile_pool(name="cps", bufs=2, space="PSUM"))
        xp = ctx.enter_context(tc.tile_pool(name="xp", bufs=3))
        ptp = ctx.enter_context(tc.tile_pool(name="ptp", bufs=3))
        smalls = ctx.enter_context(tc.tile_pool(name="smalls", bufs=4))
        scr = ctx.enter_context(tc.tile_pool(name="scr", bufs=4))
        outp = ctx.enter_context(tc.tile_pool(name="outp", bufs=3))

        # --- persistent SBUF tensors ---
        wq_sb = consts.tile([P, KSUB, DG], BF)
        wk_sb = consts.tile([P, KSUB, DG], BF)
        wv_sb = consts.tile([P, KSUB, DG], BF)
        wo_sb = consts.tile([P, DG // P, D], BF)
        # Q^T / K^T, head-pair-major: head 2*hm at [0:64, hm, :],
        # head 2*hm+1 at [64:128, hm, :].
        qt_sb = consts.tile([P, DG // P, S], BF)
        kt_sb = consts.tile([P, DG // P, S], BF)
        # V with the ones column baked in, per s-subtile and head:
        #   even h: [V(64) | 1 | 0(63)]  -> ctx rows 0-63, denom row 64
        #   odd  h: [1 | 0(63) | V(64)]  -> denom row 0, ctx rows 64-127
        v_sb = consts.tile([P, NST, HC, P], BF)
        ctxt_sb = consts.tile([P, DG // P, S], BF)  # normalized ctx^T, qt layout
        # stationaries for the recip broadcast: [1s(64)|0s] and [0s|1s(64)].
        # A matmul dst must start at partition 0, so the odd head's broadcast
        # uses a full-width stationary with zeros in the low half and
        # accumulates into the same PSUM tile as the even head's.
        ones_f = consts.tile([1, 2, P], F32)
        ones_sb = consts.tile([1, 2, P], F32R)

        xts = []  # per-slice x tiles

        def emit_xdma(n):
            xn = xp.tile([P, KSUB, SQT], BF, tag="xT", bufs=3, name=f"xn_{n}")
            for k in range(KSUB):
                nc.sync.dma_start(
                    out=xn[:, k, :], in_=xT[k * P : (k + 1) * P, n * SQT : (n + 1) * SQT]
                )
            xts.append(xn)

        # first-needed DMAs first: wq + x slice 0 gate the first matmul
        nc.sync.dma_start(out=wq_sb, in_=wq)
        emit_xdma(0)
        nc.sync.dma_start(out=wk_sb, in_=wk)
        nc.sync.dma_start(out=wv_sb, in_=wv)

        # zero/ones fills for the V padding (ones overwrite their column last)
        nc.vector.memset(v_sb[:, :, 0:HC:2, HD:P], 0.0)
        nc.vector.memset(v_sb[:, :, 1:HC:2, 0:HD], 0.0)
        for h in range(HC):
            ones_col = 64 if h % 2 == 0 else 0
            nc.vector.memset(v_sb[:, :, h, ones_col : ones_col + 1], 1.0)
        nc.vector.memset(ones_f, 0.0)
        nc.vector.memset(ones_f[:, 0, 0:64], 1.0)
        nc.vector.memset(ones_f[:, 1, 64:P], 1.0)
        nc.vector.tensor_copy(out=ones_sb, in_=ones_f)

        def proj_units(n):
            """QKV projection chunks for x slice n: 8 independent units."""
            nsl = slice(n * SQT, (n + 1) * SQT)
            units = []

            def qchunk(m):
                def u():
                    xn = xts[n]
                    ps = pps.tile([P, SQT], F32, tag="p", name=f"qps_{n}_{m}")
                    for k in range(KSUB):
                        nc.tensor.matmul(
                            ps,
                            lhsT=wq_sb[:, k, m * P : (m + 1) * P],
                            rhs=xn[:, k, :],
                            start=(k == 0),
                            stop=(k == KSUB - 1),
                        )
                    nc.vector.tensor_copy(out=qt_sb[:, m, nsl], in_=ps)
                return u

            def kchunk(m):
                def u():
                    xn = xts[n]
                    ps = pps.tile([P, SQT], F32, tag="p", name=f"kps_{n}_{m}")
                    for k in range(KSUB):
                        nc.tensor.matmul(
                            ps,
                            lhsT=wk_sb[:, k, m * P : (m + 1) * P],
                            rhs=xn[:, k, :],
                            start=(k == 0),
                            stop=(k == KSUB - 1),
                        )
                    nc.vector.tensor_copy(out=kt_sb[:, m, nsl], in_=ps)
                return u

            def vchunk(sst):
                def u():
                    xn = xts[n]
                    st0 = n * (SQT // P)
                    ps = pps.tile([P, SQT], F32, tag="p", name=f"vps_{n}_{sst}")
                    for k in range(KSUB):
                        nc.tensor.matmul(
                            ps[:, 0:DG],
                            lhsT=xn[:, k, sst * P : (sst + 1) * P],
                            rhs=wv_sb[:, k, :],
                            start=(k == 0),
                            stop=(k == KSUB - 1),
                        )
                    psv = ps[:, 0:DG].rearrange("p (h d) -> p h d", h=HC, d=HD)
                    nc.vector.tensor_copy(
                        out=v_sb[:, st0 + sst, 0:HC:2, 0:HD], in_=psv[:, 0:HC:2, :]
                    )
                    nc.vector.tensor_copy(
                        out=v_sb[:, st0 + sst, 1:HC:2, HD:P], in_=psv[:, 1:HC:2, :]
                    )
                return u

            units.append(qchunk(0))
            units.append(kchunk(0))
            units.append(qchunk(1))
            units.append(kchunk(1))
            for sst in range(SQT // P):
                units.append(vchunk(sst))
            return units

        def attn_units(n):
            """Attention steps for sq-tile n: per head pair hm, one unit per
            ski (scores even+odd packed, exp, mask, pipelined PV) plus two
            norm units."""
            nski = 4 * n + 4
            sq0 = n * SQT
            nsl = slice(sq0, sq0 + SQT)
            units = []
            def emit_pv(state, pend, hm, nski):
                ski, w0, pt = pend
                nc.tensor.matmul(
                    state["cpsA"][:, w0:],
                    lhsT=v_sb[:, ski, 2 * hm, :],
                    rhs=pt[:, w0:SQT],
                    start=(ski == 0),
                    stop=(ski == nski - 1),
                )
                nc.tensor.matmul(
                    state["cpsB"][:, w0:],
                    lhsT=v_sb[:, ski, 2 * hm + 1, :],
                    rhs=pt[:, SQT + w0 :],
                    start=(ski == 0),
                    stop=(ski == nski - 1),
                )

            for hm in range(DG // P):
                state = {"pend": None, "cpsA": None, "cpsB": None,
                         "recs": None, "scrs": None}

                def step(ski, hm=hm, state=state, n=n, nski=nski, sq0=sq0):
                    def u():
                        if ski == 0:
                            state["cpsA"] = cps.tile(
                                [P, SQT], F32, tag="ctx", name=f"cA_{n}_{hm}"
                            )
                            state["cpsB"] = cps.tile(
                                [P, SQT], F32, tag="ctx", name=f"cB_{n}_{hm}"
                            )
                        diag = ski >= 4 * n
                        w0 = (128 * ski - sq0) if diag else 0
                        spsum = sps.tile(
                            [P, 2 * SQT], F32, tag="s", name=f"s_{n}_{hm}_{ski}"
                        )
                        pt = ptp.tile(
                            [P, 2 * SQT], BF, tag="pt", name=f"pt_{n}_{hm}_{ski}"
                        )
                        # the two heads' scores: K=64 at base partitions 0 / 64,
                        # adjacent emission -> row-packed concurrent execution
                        nc.tensor.matmul(
                            spsum[:, w0:SQT],
                            lhsT=kt_sb[0:64, hm, ski * P : (ski + 1) * P],
                            rhs=qt_sb[0:64, hm, sq0 + w0 : sq0 + SQT],
                            start=True,
                            stop=True,
                        )
                        nc.tensor.matmul(
                            spsum[:, SQT + w0 :],
                            lhsT=kt_sb[64:P, hm, ski * P : (ski + 1) * P],
                            rhs=qt_sb[64:P, hm, sq0 + w0 : sq0 + SQT],
                            start=True,
                            stop=True,
                        )
                        if w0 >= 256:  # skip the big junk hole between halves
                            nc.scalar.activation(
                                out=pt[:, w0:SQT], in_=spsum[:, w0:SQT],
                                func=EXP, bias=0.0, scale=float(scale),
                            )
                            nc.scalar.activation(
                                out=pt[:, SQT + w0 :], in_=spsum[:, SQT + w0 :],
                                func=EXP, bias=0.0, scale=float(scale),
                            )
                        else:
                            nc.scalar.activation(
                                out=pt[:, w0:], in_=spsum[:, w0:],
                                func=EXP, bias=0.0, scale=float(scale),
                            )
                        if diag:  # zero entries with sk > sq in the diag block
                            for base in (0, SQT):
                                nc.gpsimd.affine_select(
                                    out=pt[:, base + w0 : base + w0 + P],
                                    in_=pt[:, base + w0 : base + w0 + P],
                                    pattern=[[1, P]],
                                    compare_op=mybir.AluOpType.is_ge,
                                    fill=0.0,
                                    base=0,
                                    channel_multiplier=-1,
                                )
                        if state["pend"] is not None:
                            emit_pv(state, state["pend"], hm, nski)
                        state["pend"] = (ski, w0, pt)
                    return u

                def normA(hm=hm, state=state, n=n, nski=nski):
                    def u():
                        emit_pv(state, state["pend"], hm, nski)
                        scrE = scr.tile([P, SQT], F32, tag="scr", name=f"scE_{n}_{hm}")
                        scrO = scr.tile([P, SQT], F32, tag="scr", name=f"scO_{n}_{hm}")
                        nc.vector.tensor_copy(out=scrE, in_=state["cpsA"])
                        nc.vector.tensor_copy(out=scrO, in_=state["cpsB"])
                        recE = smalls.tile([1, SQT], F32R, tag="rec", name=f"rE_{n}_{hm}")
                        recO = smalls.tile([1, SQT], F32R, tag="rec", name=f"rO_{n}_{hm}")
                        # f32r out is bit-identical to f32; the dtype only
                        # changes how the PE reads it as a matmul operand
                        with nc.allow_low_precision(reason="f32r view of f32"):
                            nc.vector.reciprocal(out=recE, in_=scrE[64:65, :])
                            nc.vector.reciprocal(out=recO, in_=scrO[0:1, :])
                        state["scrs"] = (scrE, scrO)
                        state["recs"] = (recE, recO)
                    return u

                def normB(hm=hm, state=state, n=n, nsl=nsl):
                    def u():
                        scrE, scrO = state["scrs"]
                        recE, recO = state["recs"]
                        bps = pps.tile([P, SQT], F32, tag="p", name=f"bc_{n}_{hm}")
                        nc.tensor.matmul(
                            bps, lhsT=ones_sb[0:1, 0, :], rhs=recE,
                            start=True, stop=False,
                        )
                        nc.tensor.matmul(
                            bps, lhsT=ones_sb[0:1, 1, :], rhs=recO,
                            start=False, stop=True,
                        )
                        nc.vector.tensor_tensor(
                            ctxt_sb[0:64, hm, nsl], scrE[0:64, :], bps[0:64, :],
                            mybir.AluOpType.mult,
                        )
                        nc.vector.tensor_tensor(
                            ctxt_sb[64:P, hm, nsl], scrO[64:P, :], bps[64:P, :],
                            mybir.AluOpType.mult,
                        )
                    return u

                for ski in range(nski):
                    units.append(step(ski))
                units.append(normA())
                units.append(normB())
            return units

        def outproj_units(n):
            """Partial output projection chunks for st tiles 4n..4n+3."""
            units = []
            ots = {}

            def chunk(st, nn):
                def u():
                    if nn == 0:
                        ots[st] = outp.tile([P, D], F32, tag="out", name=f"ot_{st}")
                    ot = ots[st]
                    ps = pps.tile([P, SQT], F32, tag="p", name=f"ops_{st}_{nn}")
                    for k in range(DG // P):
                        nc.tensor.matmul(
                            ps,
                            lhsT=ctxt_sb[:, k, st * P : (st + 1) * P],
                            rhs=wo_sb[:, k, nn * SQT : (nn + 1) * SQT],
                            start=(k == 0),
                            stop=(k == DG // P - 1),
                        )
                    nc.vector.tensor_copy(out=ot[:, nn * SQT : (nn + 1) * SQT], in_=ps)
                    if nn == D // SQT - 1:
                        nc.scalar.dma_start(out=out[st * P : (st + 1) * P, :], in_=ot)
                return u

            for st in range(4 * n, 4 * n + 4):
                for nn in range(D // SQT):
                    units.append(chunk(st, nn))
            return units

        def interleave(steps, fillers):
            """Emit steps with fillers spread evenly between them."""
            nf, ns = len(fillers), len(steps)
            fi = 0
            for i, u in enumerate(steps):
                u()
                want = (i + 1) * nf // ns
                while fi < want:
                    fillers[fi]()
                    fi += 1
            while fi < nf:
                fillers[fi]()
                fi += 1

        # --- schedule ---
        emit_xdma(1)
        for u in proj_units(0):
            u()
        emit_xdma(2)
        interleave(attn_units(0), proj_units(1))
        emit_xdma(3)
        interleave(attn_units(1), proj_units(2))
        nc.sync.dma_start(out=wo_sb, in_=wo)
        interleave(attn_units(2), proj_units(3))
        interleave(
            attn_units(3),
            outproj_units(0) + outproj_units(1) + outproj_units(2),
        )
        for u in outproj_units(3):
            u()


def build_nc():
    if "nc" in _CACHE:
        return _CACHE["nc"]
    nc = bacc.Bacc("TRN2", target_bir_lowering=False, debug=False, num_devices=NCORES)
    xT = nc.dram_tensor("xT", (D, S), BF, kind="ExternalInput").ap()
    wq = nc.dram_tensor("wq", (P, KSUB, DG), BF, kind="ExternalInput").ap()
    wk = nc.dram_tensor("wk", (P, KSUB, DG), BF, kind="ExternalInput").ap()
    wv = nc.dram_tensor("wv", (P, KSUB, DG), BF, kind="ExternalInput").ap()
    wo = nc.dram_tensor("wo", (P, DG // P, D), BF, kind="ExternalInput").ap()
    out = nc.dram_tensor("out", (S, D), F32, kind="ExternalOutput").ap()
    with tile.TileContext(nc) as tc:
        _mha_tile_kernel(tc, xT, wq, wk, wv, wo, out)
    nc.compile()
    _CACHE["nc"] = nc
    return nc


def make_in_maps(x, Wq, Wk, Wv, Wo):
    bf = ml_dtypes.bfloat16
    x = np.asarray(x, np.float32)
    in_maps = []
    for c in range(NCORES):
        b, g = c // GROUPS, c % GROUPS
        cols = slice(g * DG, (g + 1) * DG)

        def wslice(W):
            # [D, DG] -> [128, KSUB, DG] with [p, k, m] = W[k*128+p, m]
            return np.ascontiguousarray(
                np.asarray(W, np.float32)[:, cols]
                .reshape(KSUB, P, DG)
                .transpose(1, 0, 2)
                .astype(bf)
            )

        wo_c = np.ascontiguousarray(
            np.asarray(Wo, np.float32)[cols, :]
            .reshape(DG // P, P, D)
            .transpose(1, 0, 2)
            .astype(bf)
        )
        in_maps.append(
            {
                "xT": np.ascontiguousarray(x[b].T.astype(bf)),
                "wq": wslice(Wq),
                "wk": wslice(Wk),
                "wv": wslice(Wv),
                "wo": wo_c,
            }
        )
    return in_maps


def kernel(x, Wq, Wk, Wv, Wo, bo):
    nc = build_nc()
    in_maps = make_in_maps(x, Wq, Wk, Wv, Wo)
    trace = bool(int(os.environ.get("MHA_TRACE", "0")))
    if trace:
        _install_ntff_hook()
    res = run_bass_kernel_spmd(
        nc, in_maps, core_ids=list(range(NCORES)), trace=trace,
        trace_cores=list(range(NCORES)) if trace else None,
    )
    _CACHE["last_results"] = res
    bo = np.asarray(bo, np.float32)
    out = np.zeros((B, S, D), np.float32)
    for c in range(NCORES):
        out[c // GROUPS] += res.results[c]["out"]
    out += bo[None, None, :]
    return out


# revision 13
# speedup vs baseline: 1.0673x; 1.0673x over previous
"""Multi-head causal attention (B=2, S=2048, D=1024, H=16) on 8 trn2 cores.

Sharding: core c handles batch b = c // 4 and head group g = c % 4 (4 heads,
256 feature columns). Each core computes its heads' attention context and a
partial output projection (ctx_g @ Wo[rows_g]); the host sums the 4 partials
per batch and adds bo.

v3 design (all matmul operands bf16, fp32 PSUM/denominators/output):
- K^T and Q^T share the layout [128, hm, S]: head 2*hm in partitions 0:63,
  head 2*hm+1 in 64:127.  Score matmuls contract over K=64 at base partition
  0 / 64; bass auto-derives tile_position from base_partition, so the two
  heads' score matmuls are emitted adjacently and row-pack into disjoint
  halves of the PE array (concurrent execution, ~2x score throughput).
- Normalization without the DRAM-bounce broadcast: denominator row ->
  reciprocal (DVE) -> PE outer product ones[1,64] x recip[1,512] into PSUM
  -> tensor_tensor multiply.  Latency ~2us instead of ~9us of DMA hops.
- The whole kernel is emitted as fine-grained units: attention (n, hm, ski)
  steps are interleaved 1:k with projection / output-projection chunks so
  the exp (ACT engine) always has PE work to hide behind and the PE never
  idles >3.4us (HAM stays at full clock).
- All PSUM drains on DVE; ACT runs only the exps (same act-table set).
"""

import os
import sys
import types
from contextlib import ExitStack

import numpy as np
import ml_dtypes

import concourse.bacc as bacc
import concourse.bass as bass
import concourse.mybir as mybir
import concourse.tile as tile
from concourse.bass_utils import run_bass_kernel_spmd


def _install_ntff_hook():
    """The agent image's antenv lacks axon_hooks, so trn_boot's NTFF hook
    install degrades silently. Recreate the module + hook so trace=True works."""
    if "antenv.axon_hooks" in sys.modules:
        return
    try:
        mod = types.ModuleType("antenv.axon_hooks")
        holder = [None]
        mod.set_axon_ntff_profile_hook = lambda h: holder.__setitem__(0, h)
        mod.get_axon_ntff_profile_hook = lambda: holder[0]
        from trn_agent_boot.trn_boot import _ntff_profile_via_ctypes

        hook = _ntff_profile_via_ctypes("/opt/axon/libaxon_pjrt.so")
        if hook is None:
            return
        mod.set_axon_ntff_profile_hook(hook)
        sys.modules["antenv.axon_hooks"] = mod
    except Exception:
        pass

B, S, D, H, HD = 2, 2048, 1024, 16, 64
NCORES = 8
GROUPS = 4          # head groups (cores) per batch
HC = H // GROUPS    # heads per core
DG = HC * HD        # feature columns per core (256)
P = 128
KSUB = D // P       # 8 contraction subtiles for the projections
SQT = 512           # sq tile width (free dim of scores/ctx matmuls)
NSQ = S // SQT      # 4
NST = S // P        # 16 s subtiles of 128
F32 = mybir.dt.float32
F32R = mybir.dt.float32r
BF = mybir.dt.bfloat16
EXP = mybir.ActivationFunctionType.Exp

_CACHE = {}


def _mha_tile_kernel(tc, xT, wq, wk, wv, wo, out):
    nc = tc.nc
    scale = 1.0 / np.sqrt(np.float32(HD))

    with ExitStack() as ctx:
        consts = ctx.enter_context(tc.tile_pool(name="consts", bufs=1))
        # PSUM budget: pps 1-bank x2 + sps 2-bank x2 + cps 1-bank x2 = 8 banks
        pps = ctx.enter_context(tc.tile_pool(name="pps", bufs=2, space="PSUM"))
        sps = ctx.enter_context(tc.tile_pool(name="sps", bufs=2, space="PSUM"))
        cps = ctx.enter_context(tc.tile_pool(name="cps", bufs=2, space="PSUM"))
        xp = ctx.enter_context(tc.tile_pool(name="xp", bufs=3))
        ptp = ctx.enter_context(tc.tile_pool(name="ptp", bufs=3))
        smalls = ctx.enter_context(tc.tile_pool(name="smalls", bufs=4))
        scr = ctx.enter_context(tc.tile_pool(name="scr", bufs=4))
        outp = ctx.enter_context(tc.tile_pool(name="outp", bufs=3))

        # --- persistent SBUF tensors ---
        wq_sb = consts.tile([P, KSUB, DG], BF)
        wk_sb = consts.tile([P, KSUB, DG], BF)
        wv_sb = consts.tile([P, KSUB, DG], BF)
        wo_sb = consts.tile([P, DG // P, D], BF)
        # Q^T / K^T, head-pair-major: head 2*hm at [0:64, hm, :],
        # head 2*hm+1 at [64:128, hm, :].
        qt_sb = consts.tile([P, DG // P, S], BF)
        kt_sb = consts.tile([P, DG // P, S], BF)
        # V with the ones column baked in, per s-subtile and head:
        #   even h: [V(64) | 1 | 0(63)]  -> ctx rows 0-63, denom row 64
        #   odd  h: [1 | 0(63) | V(64)]  -> denom row 0, ctx rows 64-127
        v_sb = consts.tile([P, NST, HC, P], BF)
        ctxt_sb = consts.tile([P, DG // P, S], BF)  # normalized ctx^T, qt layout
        # stationaries for the recip broadcast: [1s(64)|0s] and [0s|1s(64)].
        # A matmul dst must start at partition 0, so the odd head's broadcast
        # uses a full-width stationary with zeros in the low half and
        # accumulates into the same PSUM tile as the even head's.
        ones_sb = consts.tile([1, 2, P], BF)

        xts = []  # per-slice x tiles

        def emit_xdma(n):
            xn = xp.tile([P, KSUB, SQT], BF, tag="xT", bufs=3, name=f"xn_{n}")
            for k in range(KSUB):
                nc.sync.dma_start(
                    out=xn[:, k, :], in_=xT[k * P : (k + 1) * P, n * SQT : (n + 1) * SQT]
                )
            xts.append(xn)

        # first-needed DMAs first: wq + x slice 0 gate the first matmul
        nc.sync.dma_start(out=wq_sb, in_=wq)
        emit_xdma(0)
        nc.sync.dma_start(out=wk_sb, in_=wk)
        nc.sync.dma_start(out=wv_sb, in_=wv)

        # zero/ones fills for the V padding (ones overwrite their column last)
        nc.vector.memset(v_sb[:, :, 0:HC:2, HD:P], 0.0)
        nc.vector.memset(v_sb[:, :, 1:HC:2, 0:HD], 0.0)
        for h in range(HC):
            ones_col = 64 if h % 2 == 0 else 0
            nc.vector.memset(v_sb[:, :, h, ones_col : ones_col + 1], 1.0)
        nc.vector.memset(ones_sb, 0.0)
        nc.vector.memset(ones_sb[:, 0, 0:64], 1.0)
        nc.vector.memset(ones_sb[:, 1, 64:P], 1.0)

        def proj_units(n):
            """QKV projection chunks for x slice n: 8 independent units."""
            nsl = slice(n * SQT, (n + 1) * SQT)
            units = []

            def qchunk(m):
                def u():
                    xn = xts[n]
                    ps = pps.tile([P, SQT], F32, tag="p", name=f"qps_{n}_{m}")
                    for k in range(KSUB):
                        nc.tensor.matmul(
                            ps,
                            lhsT=wq_sb[:, k, m * P : (m + 1) * P],
                            rhs=xn[:, k, :],
                            start=(k == 0),
                            stop=(k == KSUB - 1),
                        )
                    nc.vector.tensor_copy(out=qt_sb[:, m, nsl], in_=ps)
                return u

            def kchunk(m):
                def u():
                    xn = xts[n]
                    ps = pps.tile([P, SQT], F32, tag="p", name=f"kps_{n}_{m}")
                    for k in range(KSUB):
                        nc.tensor.matmul(
                            ps,
                            lhsT=wk_sb[:, k, m * P : (m + 1) * P],
                            rhs=xn[:, k, :],
                            start=(k == 0),
                            stop=(k == KSUB - 1),
                        )
                    nc.vector.tensor_copy(out=kt_sb[:, m, nsl], in_=ps)
                return u

            def vchunk(sst):
                def u():
                    xn = xts[n]
                    st0 = n * (SQT // P)
                    ps = pps.tile([P, SQT], F32, tag="p", name=f"vps_{n}_{sst}")
                    for k in range(KSUB):
                        nc.tensor.matmul(
                            ps[:, 0:DG],
                            lhsT=xn[:, k, sst * P : (sst + 1) * P],
                            rhs=wv_sb[:, k, :],
                            start=(k == 0),
                            stop=(k == KSUB - 1),
                        )
                    psv = ps[:, 0:DG].rearrange("p (h d) -> p h d", h=HC, d=HD)
                    nc.vector.tensor_copy(
                        out=v_sb[:, st0 + sst, 0:HC:2, 0:HD], in_=psv[:, 0:HC:2, :]
                    )
                    nc.vector.tensor_copy(
                        out=v_sb[:, st0 + sst, 1:HC:2, HD:P], in_=psv[:, 1:HC:2, :]
                    )
                return u

            units.append(qchunk(0))
            units.append(kchunk(0))
            units.append(qchunk(1))
            units.append(kchunk(1))
            for sst in range(SQT // P):
                units.append(vchunk(sst))
            return units

        def attn_units(n):
            """Attention steps for sq-tile n: per head pair hm, one unit per
            ski (scores even+odd packed, exp, mask, pipelined PV) plus two
            norm units."""
            nski = 4 * n + 4
            sq0 = n * SQT
            nsl = slice(sq0, sq0 + SQT)
            units = []
            def emit_pv(state, pend, hm, nski):
                ski, w0, pt = pend
                nc.tensor.matmul(
                    state["cpsA"][:, w0:],
                    lhsT=v_sb[:, ski, 2 * hm, :],
                    rhs=pt[:, w0:SQT],
                    start=(ski == 0),
                    stop=(ski == nski - 1),
                )
                nc.tensor.matmul(
                    state["cpsB"][:, w0:],
                    lhsT=v_sb[:, ski, 2 * hm + 1, :],
                    rhs=pt[:, SQT + w0 :],
                    start=(ski == 0),
                    stop=(ski == nski - 1),
                )

            for hm in range(DG // P):
                state = {"pend": None, "cpsA": None, "cpsB": None,
                         "recs": None, "scrs": None}

                def step(ski, hm=hm, state=state, n=n, nski=nski, sq0=sq0):
                    def u():
                        if ski == 0:
                            state["cpsA"] = cps.tile(
                                [P, SQT], F32, tag="ctx", name=f"cA_{n}_{hm}"
                            )
                            state["cpsB"] = cps.tile(
                                [P, SQT], F32, tag="ctx", name=f"cB_{n}_{hm}"
                            )
                        diag = ski >= 4 * n
                        w0 = (128 * ski - sq0) if diag else 0
                        spsum = sps.tile(
                            [P, 2 * SQT], F32, tag="s", name=f"s_{n}_{hm}_{ski}"
                        )
                        pt = ptp.tile(
                            [P, 2 * SQT], BF, tag="pt", name=f"pt_{n}_{hm}_{ski}"
                        )
                        # the two heads' scores: K=64 at base partitions 0 / 64,
                        # adjacent emission -> row-packed concurrent execution
                        nc.tensor.matmul(
                            spsum[:, w0:SQT],
                            lhsT=kt_sb[0:64, hm, ski * P : (ski + 1) * P],
                            rhs=qt_sb[0:64, hm, sq0 + w0 : sq0 + SQT],
                            start=True,
                            stop=True,
                        )
                        nc.tensor.matmul(
                            spsum[:, SQT + w0 :],
                            lhsT=kt_sb[64:P, hm, ski * P : (ski + 1) * P],
                            rhs=qt_sb[64:P, hm, sq0 + w0 : sq0 + SQT],
                            start=True,
                            stop=True,
                        )
                        if w0 >= 256:  # skip the big junk hole between halves
                            nc.scalar.activation(
                                out=pt[:, w0:SQT], in_=spsum[:, w0:SQT],
                                func=EXP, bias=0.0, scale=float(scale),
                            )
                            nc.scalar.activation(
                                out=pt[:, SQT + w0 :], in_=spsum[:, SQT + w0 :],
                                func=EXP, bias=0.0, scale=float(scale),
                            )
                        else:
                            nc.scalar.activation(
                                out=pt[:, w0:], in_=spsum[:, w0:],
                                func=EXP, bias=0.0, scale=float(scale),
                            )
                        if diag:  # zero entries with sk > sq in the diag block
                            for base in (0, SQT):
                                nc.gpsimd.affine_select(
                                    out=pt[:, base + w0 : base + w0 + P],
                                    in_=pt[:, base + w0 : base + w0 + P],
                                    pattern=[[1, P]],
                                    compare_op=mybir.AluOpType.is_ge,
                                    fill=0.0,
                                    base=0,
                                    channel_multiplier=-1,
                                )
                        if state["pend"] is not None:
                            emit_pv(state, state["pend"], hm, nski)
                        state["pend"] = (ski, w0, pt)
                    return u

                def normA(hm=hm, state=state, n=n, nski=nski):
                    def u():
                        emit_pv(state, state["pend"], hm, nski)
                        scrE = scr.tile([P, SQT], F32, tag="scr", name=f"scE_{n}_{hm}")
                        scrO = scr.tile([P, SQT], F32, tag="scr", name=f"scO_{n}_{hm}")
                        nc.vector.tensor_copy(out=scrE, in_=state["cpsA"])
                        nc.vector.tensor_copy(out=scrO, in_=state["cpsB"])
                        # scatter the denom rows across partitions: a [1, 512]
                        # DVE op runs on one lane (~3.3us); [128, 4] is ~26ns
                        sprE = smalls.tile([P, SQT // P], F32, tag="spr", name=f"spE_{n}_{hm}")
                        sprO = smalls.tile([P, SQT // P], F32, tag="spr", name=f"spO_{n}_{hm}")
                        nc.sync.dma_start(out=sprE, in_=scrE[64:65, :])
                        nc.sync.dma_start(out=sprO, in_=scrO[0:1, :])
                        state["scrs"] = (scrE, scrO)
                        state["sprs"] = (sprE, sprO)
                    return u

                def normA2(hm=hm, state=state, n=n):
                    def u():
                        sprE, sprO = state["sprs"]
                        rbE = smalls.tile([P, SQT // P], BF, tag="sprb", name=f"rbE_{n}_{hm}")
                        rbO = smalls.tile([P, SQT // P], BF, tag="sprb", name=f"rbO_{n}_{hm}")
                        with nc.allow_low_precision(reason="bf16 softmax denom"):
                            nc.vector.reciprocal(out=rbE, in_=sprE)
                            nc.vector.reciprocal(out=rbO, in_=sprO)
                        recE = smalls.tile([1, SQT], BF, tag="rec", name=f"rE_{n}_{hm}")
                        recO = smalls.tile([1, SQT], BF, tag="rec", name=f"rO_{n}_{hm}")
                        nc.sync.dma_start(out=recE, in_=rbE)
                        nc.sync.dma_start(out=recO, in_=rbO)
                        state["recs"] = (recE, recO)
                    return u

                def normB(hm=hm, state=state, n=n, nsl=nsl):
                    def u():
                        scrE, scrO = state["scrs"]
                        recE, recO = state["recs"]
                        bps = pps.tile([P, SQT], F32, tag="p", name=f"bc_{n}_{hm}")
                        nc.tensor.matmul(
                            bps, lhsT=ones_sb[0:1, 0, :], rhs=recE,
                            start=True, stop=False,
                        )
                        nc.tensor.matmul(
                            bps, lhsT=ones_sb[0:1, 1, :], rhs=recO,
                            start=False, stop=True,
                        )
                        nc.vector.tensor_tensor(
                            ctxt_sb[0:64, hm, nsl], scrE[0:64, :], bps[0:64, :],
                            mybir.AluOpType.mult,
                        )
                        nc.vector.tensor_tensor(
                            ctxt_sb[64:P, hm, nsl], scrO[64:P, :], bps[64:P, :],
                            mybir.AluOpType.mult,
                        )
                    return u

                for ski in range(nski):
                    units.append(step(ski))
                units.append(normA())
                units.append(normA2())
                units.append(normB())
            return units

        def outproj_units(n):
            """Partial output projection chunks for st tiles 4n..4n+3."""
            units = []
            ots = {}

            def chunk(st, nn):
                def u():
                    if nn == 0:
                        ots[st] = outp.tile([P, D], F32, tag="out", name=f"ot_{st}")
                    ot = ots[st]
                    ps = pps.tile([P, SQT], F32, tag="p", name=f"ops_{st}_{nn}")
                    for k in range(DG // P):
                        nc.tensor.matmul(
                            ps,
                            lhsT=ctxt_sb[:, k, st * P : (st + 1) * P],
                            rhs=wo_sb[:, k, nn * SQT : (nn + 1) * SQT],
                            start=(k == 0),
                            stop=(k == DG // P - 1),
                        )
                    nc.vector.tensor_copy(out=ot[:, nn * SQT : (nn + 1) * SQT], in_=ps)
                    if nn == D // SQT - 1:
                        nc.scalar.dma_start(out=out[st * P : (st + 1) * P, :], in_=ot)
                return u

            for st in range(4 * n, 4 * n + 4):
                for nn in range(D // SQT):
                    units.append(chunk(st, nn))
            return units

        def interleave(steps, fillers):
            """Emit steps with fillers spread evenly between them."""
            nf, ns = len(fillers), len(steps)
            fi = 0
            for i, u in enumerate(steps):
                u()
                want = (i + 1) * nf // ns
                while fi < want:
                    fillers[fi]()
                    fi += 1
            while fi < nf:
                fillers[fi]()
                fi += 1

        # --- schedule ---
        emit_xdma(1)
        for u in proj_units(0):
            u()
        emit_xdma(2)
        interleave(attn_units(0), proj_units(1))
        emit_xdma(3)
        interleave(attn_units(1), proj_units(2))
        nc.sync.dma_start(out=wo_sb, in_=wo)
        interleave(attn_units(2), proj_units(3))
        interleave(
            attn_units(3),
            outproj_units(0) + outproj_units(1) + outproj_units(2),
        )
        for u in outproj_units(3):
            u()


def build_nc():
    if "nc" in _CACHE:
        return _CACHE["nc"]
    nc = bacc.Bacc("TRN2", target_bir_lowering=False, debug=False, num_devices=NCORES)
    xT = nc.dram_tensor("xT", (D, S), BF, kind="ExternalInput").ap()
    wq = nc.dram_tensor("wq", (P, KSUB, DG), BF, kind="ExternalInput").ap()
    wk = nc.dram_tensor("wk", (P, KSUB, DG), BF, kind="ExternalInput").ap()
    wv = nc.dram_tensor("wv", (P, KSUB, DG), BF, kind="ExternalInput").ap()
    wo = nc.dram_tensor("wo", (P, DG // P, D), BF, kind="ExternalInput").ap()
    out = nc.dram_tensor("out", (S, D), F32, kind="ExternalOutput").ap()
    with tile.TileContext(nc) as tc:
        _mha_tile_kernel(tc, xT, wq, wk, wv, wo, out)
    nc.compile()
    _CACHE["nc"] = nc
    return nc


def make_in_maps(x, Wq, Wk, Wv, Wo):
    bf = ml_dtypes.bfloat16
    x = np.asarray(x, np.float32)
    in_maps = []
    for c in range(NCORES):
        b, g = c // GROUPS, c % GROUPS
        cols = slice(g * DG, (g + 1) * DG)

        def wslice(W):
            # [D, DG] -> [128, KSUB, DG] with [p, k, m] = W[k*128+p, m]
            return np.ascontiguousarray(
                np.asarray(W, np.float32)[:, cols]
                .reshape(KSUB, P, DG)
                .transpose(1, 0, 2)
                .astype(bf)
            )

        wo_c = np.ascontiguousarray(
            np.asarray(Wo, np.float32)[cols, :]
            .reshape(DG // P, P, D)
            .transpose(1, 0, 2)
            .astype(bf)
        )
        in_maps.append(
            {
                "xT": np.ascontiguousarray(x[b].T.astype(bf)),
                "wq": wslice(Wq),
                "wk": wslice(Wk),
                "wv": wslice(Wv),
                "wo": wo_c,
            }
        )
    return in_maps


def kernel(x, Wq, Wk, Wv, Wo, bo):
    nc = build_nc()
    in_maps = make_in_maps(x, Wq, Wk, Wv, Wo)
    trace = bool(int(os.environ.get("MHA_TRACE", "0")))
    if trace:
        _install_ntff_hook()
    res = run_bass_kernel_spmd(
        nc, in_maps, core_ids=list(range(NCORES)), trace=trace,
        trace_cores=list(range(NCORES)) if trace else None,
    )
    _CACHE["last_results"] = res
    bo = np.asarray(bo, np.float32)
    out = np.zeros((B, S, D), np.float32)
    for c in range(NCORES):
        out[c // GROUPS] += res.results[c]["out"]
    out += bo[None, None, :]
    return out


# revision 15
# speedup vs baseline: 1.3036x; 1.2214x over previous
"""Multi-head causal attention (B=2, S=2048, D=1024, H=16) on 8 trn2 cores.

Sharding: core c handles batch b = c // 4 and head group g = c % 4 (4 heads,
256 feature columns). Each core computes its heads' attention context and a
partial output projection (ctx_g @ Wo[rows_g]); the host sums the 4 partials
per batch and adds bo.

v3 design (all matmul operands bf16, fp32 PSUM/denominators/output):
- K^T and Q^T share the layout [128, hm, S]: head 2*hm in partitions 0:63,
  head 2*hm+1 in 64:127.  Score matmuls contract over K=64 at base partition
  0 / 64; bass auto-derives tile_position from base_partition, so the two
  heads' score matmuls are emitted adjacently and row-pack into disjoint
  halves of the PE array (concurrent execution, ~2x score throughput).
- Normalization without the DRAM-bounce broadcast: denominator row ->
  reciprocal (DVE) -> PE outer product ones[1,64] x recip[1,512] into PSUM
  -> tensor_tensor multiply.  Latency ~2us instead of ~9us of DMA hops.
- The whole kernel is emitted as fine-grained units: attention (n, hm, ski)
  steps are interleaved 1:k with projection / output-projection chunks so
  the exp (ACT engine) always has PE work to hide behind and the PE never
  idles >3.4us (HAM stays at full clock).
- All PSUM drains on DVE; ACT runs only the exps (same act-table set).
"""

import os
import sys
import types
from contextlib import ExitStack

import numpy as np
import ml_dtypes

import concourse.bacc as bacc
import concourse.bass as bass
import concourse.mybir as mybir
import concourse.tile as tile
from concourse.bass_utils import run_bass_kernel_spmd


def _install_ntff_hook():
    """The agent image's antenv lacks axon_hooks, so trn_boot's NTFF hook
    install degrades silently. Recreate the module + hook so trace=True works."""
    if "antenv.axon_hooks" in sys.modules:
        return
    try:
        mod = types.ModuleType("antenv.axon_hooks")
        holder = [None]
        mod.set_axon_ntff_profile_hook = lambda h: holder.__setitem__(0, h)
        mod.get_axon_ntff_profile_hook = lambda: holder[0]
        from trn_agent_boot.trn_boot import _ntff_profile_via_ctypes

        hook = _ntff_profile_via_ctypes("/opt/axon/libaxon_pjrt.so")
        if hook is None:
            return
        mod.set_axon_ntff_profile_hook(hook)
        sys.modules["antenv.axon_hooks"] = mod
    except Exception:
        pass

B, S, D, H, HD = 2, 2048, 1024, 16, 64
NCORES = 8
GROUPS = 4          # head groups (cores) per batch
HC = H // GROUPS    # heads per core
DG = HC * HD        # feature columns per core (256)
P = 128
KSUB = D // P       # 8 contraction subtiles for the projections
SQT = 512           # sq tile width (free dim of scores/ctx matmuls)
NSQ = S // SQT      # 4
NST = S // P        # 16 s subtiles of 128
F32 = mybir.dt.float32
F32R = mybir.dt.float32r
BF = mybir.dt.bfloat16
EXP = mybir.ActivationFunctionType.Exp

_CACHE = {}


def _mha_tile_kernel(tc, xT, wq, wk, wv, wo, out):
    nc = tc.nc
    scale = 1.0 / np.sqrt(np.float32(HD))

    with ExitStack() as ctx:
        consts = ctx.enter_context(tc.tile_pool(name="consts", bufs=1))
        # PSUM budget: pps 1-bank x2 + sps 2-bank x2 + cps 1-bank x2 = 8 banks
        pps = ctx.enter_context(tc.tile_pool(name="pps", bufs=2, space="PSUM"))
        sps = ctx.enter_context(tc.tile_pool(name="sps", bufs=2, space="PSUM"))
        cps = ctx.enter_context(tc.tile_pool(name="cps", bufs=2, space="PSUM"))
        xp = ctx.enter_context(tc.tile_pool(name="xp", bufs=3))
        ptp = ctx.enter_context(tc.tile_pool(name="ptp", bufs=3))
        smalls = ctx.enter_context(tc.tile_pool(name="smalls", bufs=4))
        scr = ctx.enter_context(tc.tile_pool(name="scr", bufs=4))
        outp = ctx.enter_context(tc.tile_pool(name="outp", bufs=3))

        # --- persistent SBUF tensors ---
        wq_sb = consts.tile([P, KSUB, DG], BF)
        wk_sb = consts.tile([P, KSUB, DG], BF)
        wv_sb = consts.tile([P, KSUB, DG], BF)
        wo_sb = consts.tile([P, DG // P, D], BF)
        # Q^T, head-pair-major: head 2*hm at [0:64, hm, :], head 2*hm+1 at
        # [64:128, hm, :].  K^T zero-padded per head (head h's 64 rows at
        # [64*(h%2):, h, :], rest 0) so score matmuls contract over K=128 --
        # full-row weights keep Fast Weight Load enabled (K=64 stationaries
        # disable FWL and expose ~100ns of LDWEIGHTS per matmul).
        qt_sb = consts.tile([P, DG // P, S], BF)
        kt_sb = consts.tile([P, HC, S], BF)
        # V with the ones column baked in, per s-subtile and head:
        #   even h: [V(64) | 1 | 0(63)]  -> ctx rows 0-63, denom row 64
        #   odd  h: [1 | 0(63) | V(64)]  -> denom row 0, ctx rows 64-127
        v_sb = consts.tile([P, NST, HC, P], BF)
        ctxt_sb = consts.tile([P, DG // P, S], BF)  # normalized ctx^T, qt layout
        # stationaries for the recip broadcast: [1s(64)|0s] and [0s|1s(64)].
        # A matmul dst must start at partition 0, so the odd head's broadcast
        # uses a full-width stationary with zeros in the low half and
        # accumulates into the same PSUM tile as the even head's.
        ones_sb = consts.tile([1, 2, P], BF)

        xts = []  # per-slice x tiles

        def emit_xdma(n):
            xn = xp.tile([P, KSUB, SQT], BF, tag="xT", bufs=3, name=f"xn_{n}")
            for k in range(KSUB):
                nc.sync.dma_start(
                    out=xn[:, k, :], in_=xT[k * P : (k + 1) * P, n * SQT : (n + 1) * SQT]
                )
            xts.append(xn)

        # first-needed DMAs first: wq + x slice 0 gate the first matmul
        nc.sync.dma_start(out=wq_sb, in_=wq)
        emit_xdma(0)
        nc.sync.dma_start(out=wk_sb, in_=wk)
        nc.sync.dma_start(out=wv_sb, in_=wv)

        # zero/ones fills for the K^T / V padding
        nc.vector.memset(kt_sb[64:P, 0:HC:2, :], 0.0)
        nc.vector.memset(kt_sb[0:64, 1:HC:2, :], 0.0)
        nc.vector.memset(v_sb[:, :, 0:HC:2, HD:P], 0.0)
        nc.vector.memset(v_sb[:, :, 1:HC:2, 0:HD], 0.0)
        for h in range(HC):
            ones_col = 64 if h % 2 == 0 else 0
            nc.vector.memset(v_sb[:, :, h, ones_col : ones_col + 1], 1.0)
        nc.vector.memset(ones_sb, 0.0)
        nc.vector.memset(ones_sb[:, 0, 0:64], 1.0)
        nc.vector.memset(ones_sb[:, 1, 64:P], 1.0)

        def proj_units(n):
            """QKV projection chunks for x slice n: 8 independent units."""
            nsl = slice(n * SQT, (n + 1) * SQT)
            units = []

            def qchunk(m):
                def u():
                    xn = xts[n]
                    ps = pps.tile([P, SQT], F32, tag="p", name=f"qps_{n}_{m}")
                    for k in range(KSUB):
                        nc.tensor.matmul(
                            ps,
                            lhsT=wq_sb[:, k, m * P : (m + 1) * P],
                            rhs=xn[:, k, :],
                            start=(k == 0),
                            stop=(k == KSUB - 1),
                        )
                    nc.vector.tensor_copy(out=qt_sb[:, m, nsl], in_=ps)
                return u

            def kchunk(m):
                def u():
                    xn = xts[n]
                    ps = pps.tile([P, SQT], F32, tag="p", name=f"kps_{n}_{m}")
                    for k in range(KSUB):
                        nc.tensor.matmul(
                            ps,
                            lhsT=wk_sb[:, k, m * P : (m + 1) * P],
                            rhs=xn[:, k, :],
                            start=(k == 0),
                            stop=(k == KSUB - 1),
                        )
                    nc.vector.tensor_copy(out=kt_sb[0:64, 2 * m, nsl], in_=ps[0:64, :])
                    nc.vector.tensor_copy(
                        out=kt_sb[64:P, 2 * m + 1, nsl], in_=ps[64:P, :]
                    )
                return u

            def vchunk(sst):
                def u():
                    xn = xts[n]
                    st0 = n * (SQT // P)
                    ps = pps.tile([P, SQT], F32, tag="p", name=f"vps_{n}_{sst}")
                    for k in range(KSUB):
                        nc.tensor.matmul(
                            ps[:, 0:DG],
                            lhsT=xn[:, k, sst * P : (sst + 1) * P],
                            rhs=wv_sb[:, k, :],
                            start=(k == 0),
                            stop=(k == KSUB - 1),
                        )
                    psv = ps[:, 0:DG].rearrange("p (h d) -> p h d", h=HC, d=HD)
                    nc.vector.tensor_copy(
                        out=v_sb[:, st0 + sst, 0:HC:2, 0:HD], in_=psv[:, 0:HC:2, :]
                    )
                    nc.vector.tensor_copy(
                        out=v_sb[:, st0 + sst, 1:HC:2, HD:P], in_=psv[:, 1:HC:2, :]
                    )
                return u

            units.append(qchunk(0))
            units.append(kchunk(0))
            units.append(qchunk(1))
            units.append(kchunk(1))
            for sst in range(SQT // P):
                units.append(vchunk(sst))
            return units

        def attn_units(n):
            """Attention steps for sq-tile n: per head pair hm, one unit per
            ski (scores even+odd packed, exp, mask, pipelined PV) plus two
            norm units."""
            nski = 4 * n + 4
            sq0 = n * SQT
            nsl = slice(sq0, sq0 + SQT)
            units = []
            def emit_pv(state, pend, hm, nski):
                ski, w0, pt = pend
                nc.tensor.matmul(
                    state["cpsA"][:, w0:],
                    lhsT=v_sb[:, ski, 2 * hm, :],
                    rhs=pt[:, w0:SQT],
                    start=(ski == 0),
                    stop=(ski == nski - 1),
                )
                nc.tensor.matmul(
                    state["cpsB"][:, w0:],
                    lhsT=v_sb[:, ski, 2 * hm + 1, :],
                    rhs=pt[:, SQT + w0 :],
                    start=(ski == 0),
                    stop=(ski == nski - 1),
                )

            for hm in range(DG // P):
                state = {"pend": None, "cpsA": None, "cpsB": None,
                         "recs": None, "scrs": None}

                def step(ski, hm=hm, state=state, n=n, nski=nski, sq0=sq0):
                    def u():
                        if ski == 0:
                            state["cpsA"] = cps.tile(
                                [P, SQT], F32, tag="ctx", name=f"cA_{n}_{hm}"
                            )
                            state["cpsB"] = cps.tile(
                                [P, SQT], F32, tag="ctx", name=f"cB_{n}_{hm}"
                            )
                        diag = ski >= 4 * n
                        w0 = (128 * ski - sq0) if diag else 0
                        spsum = sps.tile(
                            [P, 2 * SQT], F32, tag="s", name=f"s_{n}_{hm}_{ski}"
                        )
                        pt = ptp.tile(
                            [P, 2 * SQT], BF, tag="pt", name=f"pt_{n}_{hm}_{ski}"
                        )
                        # the two heads' scores; kt zero-padding makes the
                        # other head's q rows contract to 0
                        nc.tensor.matmul(
                            spsum[:, w0:SQT],
                            lhsT=kt_sb[:, 2 * hm, ski * P : (ski + 1) * P],
                            rhs=qt_sb[:, hm, sq0 + w0 : sq0 + SQT],
                            start=True,
                            stop=True,
                        )
                        nc.tensor.matmul(
                            spsum[:, SQT + w0 :],
                            lhsT=kt_sb[:, 2 * hm + 1, ski * P : (ski + 1) * P],
                            rhs=qt_sb[:, hm, sq0 + w0 : sq0 + SQT],
                            start=True,
                            stop=True,
                        )
                        if w0 >= 256:  # skip the big junk hole between halves
                            nc.scalar.activation(
                                out=pt[:, w0:SQT], in_=spsum[:, w0:SQT],
                                func=EXP, bias=0.0, scale=float(scale),
                            )
                            nc.scalar.activation(
                                out=pt[:, SQT + w0 :], in_=spsum[:, SQT + w0 :],
                                func=EXP, bias=0.0, scale=float(scale),
                            )
                        else:
                            nc.scalar.activation(
                                out=pt[:, w0:], in_=spsum[:, w0:],
                                func=EXP, bias=0.0, scale=float(scale),
                            )
                        if diag:  # zero entries with sk > sq in the diag block
                            for base in (0, SQT):
                                nc.gpsimd.affine_select(
                                    out=pt[:, base + w0 : base + w0 + P],
                                    in_=pt[:, base + w0 : base + w0 + P],
                                    pattern=[[1, P]],
                                    compare_op=mybir.AluOpType.is_ge,
                                    fill=0.0,
                                    base=0,
                                    channel_multiplier=-1,
                                )
                        if state["pend"] is not None:
                            emit_pv(state, state["pend"], hm, nski)
                        state["pend"] = (ski, w0, pt)
                    return u

                def normA(hm=hm, state=state, n=n, nski=nski):
                    def u():
                        emit_pv(state, state["pend"], hm, nski)
                        scrE = scr.tile([P, SQT], F32, tag="scr", name=f"scE_{n}_{hm}")
                        scrO = scr.tile([P, SQT], F32, tag="scr", name=f"scO_{n}_{hm}")
                        nc.vector.tensor_copy(out=scrE, in_=state["cpsA"])
                        nc.vector.tensor_copy(out=scrO, in_=state["cpsB"])
                        # scatter the denom rows across partitions: a [1, 512]
                        # DVE op runs on one lane (~3.3us); [128, 4] is ~26ns
                        sprE = smalls.tile([P, SQT // P], F32, tag="spr", name=f"spE_{n}_{hm}")
                        sprO = smalls.tile([P, SQT // P], F32, tag="spr", name=f"spO_{n}_{hm}")
                        nc.sync.dma_start(out=sprE, in_=scrE[64:65, :])
                        nc.sync.dma_start(out=sprO, in_=scrO[0:1, :])
                        state["scrs"] = (scrE, scrO)
                        state["sprs"] = (sprE, sprO)
                    return u

                def normA2(hm=hm, state=state, n=n):
                    def u():
                        sprE, sprO = state["sprs"]
                        rbE = smalls.tile([P, SQT // P], BF, tag="sprb", name=f"rbE_{n}_{hm}")
                        rbO = smalls.tile([P, SQT // P], BF, tag="sprb", name=f"rbO_{n}_{hm}")
                        with nc.allow_low_precision(reason="bf16 softmax denom"):
                            nc.vector.reciprocal(out=rbE, in_=sprE)
                            nc.vector.reciprocal(out=rbO, in_=sprO)
                        recE = smalls.tile([1, SQT], BF, tag="rec", name=f"rE_{n}_{hm}")
                        recO = smalls.tile([1, SQT], BF, tag="rec", name=f"rO_{n}_{hm}")
                        nc.sync.dma_start(out=recE, in_=rbE)
                        nc.sync.dma_start(out=recO, in_=rbO)
                        state["recs"] = (recE, recO)
                    return u

                def normB(hm=hm, state=state, n=n, nsl=nsl):
                    def u():
                        scrE, scrO = state["scrs"]
                        recE, recO = state["recs"]
                        bps = pps.tile([P, SQT], F32, tag="p", name=f"bc_{n}_{hm}")
                        nc.tensor.matmul(
                            bps, lhsT=ones_sb[0:1, 0, :], rhs=recE,
                            start=True, stop=False,
                        )
                        nc.tensor.matmul(
                            bps, lhsT=ones_sb[0:1, 1, :], rhs=recO,
                            start=False, stop=True,
                        )
                        nc.vector.tensor_tensor(
                            ctxt_sb[0:64, hm, nsl], scrE[0:64, :], bps[0:64, :],
                            mybir.AluOpType.mult,
                        )
                        nc.vector.tensor_tensor(
                            ctxt_sb[64:P, hm, nsl], scrO[64:P, :], bps[64:P, :],
                            mybir.AluOpType.mult,
                        )
                    return u

                for ski in range(nski):
                    units.append(step(ski))
                units.append(normA())
                units.append(normA2())
                units.append(normB())
            return units

        def outproj_units(n):
            """Partial output projection chunks for st tiles 4n..4n+3."""
            units = []
            ots = {}

            def chunk(st, nn):
                def u():
                    if nn == 0:
                        ots[st] = outp.tile([P, D], BF, tag="out", name=f"ot_{st}")
                    ot = ots[st]
                    ps = pps.tile([P, SQT], F32, tag="p", name=f"ops_{st}_{nn}")
                    for k in range(DG // P):
                        nc.tensor.matmul(
                            ps,
                            lhsT=ctxt_sb[:, k, st * P : (st + 1) * P],
                            rhs=wo_sb[:, k, nn * SQT : (nn + 1) * SQT],
                            start=(k == 0),
                            stop=(k == DG // P - 1),
                        )
                    nc.vector.tensor_copy(out=ot[:, nn * SQT : (nn + 1) * SQT], in_=ps)
                    if nn == D // SQT - 1:
                        eng = (nc.scalar, nc.gpsimd)[st % 2]
                        eng.dma_start(out=out[st * P : (st + 1) * P, :], in_=ot)
                return u

            for st in range(4 * n, 4 * n + 4):
                for nn in range(D // SQT):
                    units.append(chunk(st, nn))
            return units

        def interleave(steps, fillers, reserve=0):
            """Emit steps with fillers spread evenly between them; the last
            `reserve` fillers are held back until after all steps (PE work to
            hide the final norm-chain latency)."""
            nf, ns = len(fillers) - reserve, len(steps)
            fi = 0
            for i, u in enumerate(steps):
                u()
                want = (i + 1) * nf // ns
                while fi < want:
                    fillers[fi]()
                    fi += 1
            while fi < len(fillers):
                fillers[fi]()
                fi += 1

        # --- schedule ---
        emit_xdma(1)
        for u in proj_units(0):
            u()
        emit_xdma(2)
        interleave(attn_units(0), proj_units(1))
        emit_xdma(3)
        interleave(attn_units(1), proj_units(2))
        nc.sync.dma_start(out=wo_sb, in_=wo)
        interleave(attn_units(2), proj_units(3))
        interleave(
            attn_units(3),
            outproj_units(0) + outproj_units(1) + outproj_units(2),
            reserve=4,
        )
        for u in outproj_units(3):
            u()


def build_nc():
    if "nc" in _CACHE:
        return _CACHE["nc"]
    nc = bacc.Bacc("TRN2", target_bir_lowering=False, debug=False, num_devices=NCORES)
    xT = nc.dram_tensor("xT", (D, S), BF, kind="ExternalInput").ap()
    wq = nc.dram_tensor("wq", (P, KSUB, DG), BF, kind="ExternalInput").ap()
    wk = nc.dram_tensor("wk", (P, KSUB, DG), BF, kind="ExternalInput").ap()
    wv = nc.dram_tensor("wv", (P, KSUB, DG), BF, kind="ExternalInput").ap()
    wo = nc.dram_tensor("wo", (P, DG // P, D), BF, kind="ExternalInput").ap()
    out = nc.dram_tensor("out", (S, D), BF, kind="ExternalOutput").ap()
    with tile.TileContext(nc) as tc:
        _mha_tile_kernel(tc, xT, wq, wk, wv, wo, out)
    nc.compile()
    _CACHE["nc"] = nc
    return nc


def make_in_maps(x, Wq, Wk, Wv, Wo):
    bf = ml_dtypes.bfloat16
    x = np.asarray(x, np.float32)
    in_maps = []
    for c in range(NCORES):
        b, g = c // GROUPS, c % GROUPS
        cols = slice(g * DG, (g + 1) * DG)

        def wslice(W):
            # [D, DG] -> [128, KSUB, DG] with [p, k, m] = W[k*128+p, m]
            return np.ascontiguousarray(
                np.asarray(W, np.float32)[:, cols]
                .reshape(KSUB, P, DG)
                .transpose(1, 0, 2)
                .astype(bf)
            )

        wo_c = np.ascontiguousarray(
            np.asarray(Wo, np.float32)[cols, :]
            .reshape(DG // P, P, D)
            .transpose(1, 0, 2)
            .astype(bf)
        )
        in_maps.append(
            {
                "xT": np.ascontiguousarray(x[b].T.astype(bf)),
                "wq": wslice(Wq),
                "wk": wslice(Wk),
                "wv": wslice(Wv),
                "wo": wo_c,
            }
        )
    return in_maps


def kernel(x, Wq, Wk, Wv, Wo, bo):
    nc = build_nc()
    in_maps = make_in_maps(x, Wq, Wk, Wv, Wo)
    trace = bool(int(os.environ.get("MHA_TRACE", "0")))
    if trace:
        _install_ntff_hook()
    res = run_bass_kernel_spmd(
        nc, in_maps, core_ids=list(range(NCORES)), trace=trace,
        trace_cores=list(range(NCORES)) if trace else None,
    )
    _CACHE["last_results"] = res
    bo = np.asarray(bo, np.float32)
    out = np.zeros((B, S, D), np.float32)
    for c in range(NCORES):
        out[c // GROUPS] += np.asarray(res.results[c]["out"], np.float32)
    out += bo[None, None, :]
    return out


# revision 16
# speedup vs baseline: 1.3494x; 1.0351x over previous
"""Multi-head causal attention (B=2, S=2048, D=1024, H=16) on 8 trn2 cores.

Sharding: core c handles batch b = c // 4 and head group g = c % 4 (4 heads,
256 feature columns). Each core computes its heads' attention context and a
partial output projection (ctx_g @ Wo[rows_g]); the host sums the 4 partials
per batch and adds bo.

v3 design (all matmul operands bf16, fp32 PSUM/denominators/output):
- K^T and Q^T share the layout [128, hm, S]: head 2*hm in partitions 0:63,
  head 2*hm+1 in 64:127.  Score matmuls contract over K=64 at base partition
  0 / 64; bass auto-derives tile_position from base_partition, so the two
  heads' score matmuls are emitted adjacently and row-pack into disjoint
  halves of the PE array (concurrent execution, ~2x score throughput).
- Normalization without the DRAM-bounce broadcast: denominator row ->
  reciprocal (DVE) -> PE outer product ones[1,64] x recip[1,512] into PSUM
  -> tensor_tensor multiply.  Latency ~2us instead of ~9us of DMA hops.
- The whole kernel is emitted as fine-grained units: attention (n, hm, ski)
  steps are interleaved 1:k with projection / output-projection chunks so
  the exp (ACT engine) always has PE work to hide behind and the PE never
  idles >3.4us (HAM stays at full clock).
- All PSUM drains on DVE; ACT runs only the exps (same act-table set).
"""

import os
import sys
import types
from contextlib import ExitStack

import numpy as np
import ml_dtypes

import concourse.bacc as bacc
import concourse.bass as bass
import concourse.mybir as mybir
import concourse.tile as tile
from concourse.bass_utils import run_bass_kernel_spmd


def _install_ntff_hook():
    """The agent image's antenv lacks axon_hooks, so trn_boot's NTFF hook
    install degrades silently. Recreate the module + hook so trace=True works."""
    if "antenv.axon_hooks" in sys.modules:
        return
    try:
        mod = types.ModuleType("antenv.axon_hooks")
        holder = [None]
        mod.set_axon_ntff_profile_hook = lambda h: holder.__setitem__(0, h)
        mod.get_axon_ntff_profile_hook = lambda: holder[0]
        from trn_agent_boot.trn_boot import _ntff_profile_via_ctypes

        hook = _ntff_profile_via_ctypes("/opt/axon/libaxon_pjrt.so")
        if hook is None:
            return
        mod.set_axon_ntff_profile_hook(hook)
        sys.modules["antenv.axon_hooks"] = mod
    except Exception:
        pass

B, S, D, H, HD = 2, 2048, 1024, 16, 64
NCORES = 8
GROUPS = 4          # head groups (cores) per batch
HC = H // GROUPS    # heads per core
DG = HC * HD        # feature columns per core (256)
P = 128
KSUB = D // P       # 8 contraction subtiles for the projections
SQT = 512           # sq tile width (free dim of scores/ctx matmuls)
NSQ = S // SQT      # 4
NST = S // P        # 16 s subtiles of 128
F32 = mybir.dt.float32
F32R = mybir.dt.float32r
BF = mybir.dt.bfloat16
EXP = mybir.ActivationFunctionType.Exp

_CACHE = {}


def _mha_tile_kernel(tc, xT, wq, wk, wv, wo, out):
    nc = tc.nc
    scale = 1.0 / np.sqrt(np.float32(HD))

    with ExitStack() as ctx:
        consts = ctx.enter_context(tc.tile_pool(name="consts", bufs=1))
        # PSUM budget: pps 1-bank x2 + sps 2-bank x2 + cps 1-bank x2 = 8 banks
        pps = ctx.enter_context(tc.tile_pool(name="pps", bufs=2, space="PSUM"))
        sps = ctx.enter_context(tc.tile_pool(name="sps", bufs=2, space="PSUM"))
        cps = ctx.enter_context(tc.tile_pool(name="cps", bufs=2, space="PSUM"))
        xp = ctx.enter_context(tc.tile_pool(name="xp", bufs=3))
        ptp = ctx.enter_context(tc.tile_pool(name="ptp", bufs=3))
        smalls = ctx.enter_context(tc.tile_pool(name="smalls", bufs=4))
        scr = ctx.enter_context(tc.tile_pool(name="scr", bufs=4))
        outp = ctx.enter_context(tc.tile_pool(name="outp", bufs=3))

        # --- persistent SBUF tensors ---
        wq_sb = consts.tile([P, KSUB, DG], BF)
        wk_sb = consts.tile([P, KSUB, DG], BF)
        wv_sb = consts.tile([P, KSUB, DG], BF)
        wo_sb = consts.tile([P, DG // P, D], BF)
        # Q^T, head-pair-major: head 2*hm at [0:64, hm, :], head 2*hm+1 at
        # [64:128, hm, :].  K^T zero-padded per head (head h's 64 rows at
        # [64*(h%2):, h, :], rest 0) so score matmuls contract over K=128 --
        # full-row weights keep Fast Weight Load enabled (K=64 stationaries
        # disable FWL and expose ~100ns of LDWEIGHTS per matmul).
        qt_sb = consts.tile([P, DG // P, S], BF)
        kt_sb = consts.tile([P, HC, S], BF)
        # V with the ones column baked in, per s-subtile and head:
        #   even h: [V(64) | 1 | 0(63)]  -> ctx rows 0-63, denom row 64
        #   odd  h: [1 | 0(63) | V(64)]  -> denom row 0, ctx rows 64-127
        v_sb = consts.tile([P, NST, HC, P], BF)
        ctxt_sb = consts.tile([P, DG // P, S], BF)  # normalized ctx^T, qt layout
        # stationaries for the recip broadcast: [1s(64)|0s] and [0s|1s(64)].
        # A matmul dst must start at partition 0, so the odd head's broadcast
        # uses a full-width stationary with zeros in the low half and
        # accumulates into the same PSUM tile as the even head's.
        ones_sb = consts.tile([1, 2, P], BF)

        xts = []  # per-slice x tiles

        def emit_xdma(n):
            xn = xp.tile([P, KSUB, SQT], BF, tag="xT", bufs=3, name=f"xn_{n}")
            for k in range(KSUB):
                nc.sync.dma_start(
                    out=xn[:, k, :], in_=xT[k * P : (k + 1) * P, n * SQT : (n + 1) * SQT]
                )
            xts.append(xn)

        # first-needed DMAs first: wq + x slice 0 gate the first matmul
        nc.sync.dma_start(out=wq_sb, in_=wq)
        emit_xdma(0)
        nc.sync.dma_start(out=wk_sb, in_=wk)
        nc.sync.dma_start(out=wv_sb, in_=wv)

        # zero/ones fills for the K^T / V padding
        nc.vector.memset(kt_sb[64:P, 0:HC:2, :], 0.0)
        nc.vector.memset(kt_sb[0:64, 1:HC:2, :], 0.0)
        nc.vector.memset(v_sb[:, :, 0:HC:2, HD:P], 0.0)
        nc.vector.memset(v_sb[:, :, 1:HC:2, 0:HD], 0.0)
        for h in range(HC):
            ones_col = 64 if h % 2 == 0 else 0
            nc.vector.memset(v_sb[:, :, h, ones_col : ones_col + 1], 1.0)
        nc.vector.memset(ones_sb, 0.0)
        nc.vector.memset(ones_sb[:, 0, 0:64], 1.0)
        nc.vector.memset(ones_sb[:, 1, 64:P], 1.0)

        def proj_units(n):
            """QKV projection chunks for x slice n: 8 independent units."""
            nsl = slice(n * SQT, (n + 1) * SQT)
            units = []

            def qchunk(m):
                def u():
                    xn = xts[n]
                    ps = pps.tile([P, SQT], F32, tag="p", name=f"qps_{n}_{m}")
                    for k in range(KSUB):
                        nc.tensor.matmul(
                            ps,
                            lhsT=wq_sb[:, k, m * P : (m + 1) * P],
                            rhs=xn[:, k, :],
                            start=(k == 0),
                            stop=(k == KSUB - 1),
                        )
                    nc.vector.tensor_copy(out=qt_sb[:, m, nsl], in_=ps)
                return u

            def kchunk(m):
                def u():
                    xn = xts[n]
                    ps = pps.tile([P, SQT], F32, tag="p", name=f"kps_{n}_{m}")
                    for k in range(KSUB):
                        nc.tensor.matmul(
                            ps,
                            lhsT=wk_sb[:, k, m * P : (m + 1) * P],
                            rhs=xn[:, k, :],
                            start=(k == 0),
                            stop=(k == KSUB - 1),
                        )
                    nc.vector.tensor_copy(out=kt_sb[0:64, 2 * m, nsl], in_=ps[0:64, :])
                    nc.vector.tensor_copy(
                        out=kt_sb[64:P, 2 * m + 1, nsl], in_=ps[64:P, :]
                    )
                return u

            def vchunk(sst):
                def u():
                    xn = xts[n]
                    st0 = n * (SQT // P)
                    ps = pps.tile([P, SQT], F32, tag="p", name=f"vps_{n}_{sst}")
                    for k in range(KSUB):
                        nc.tensor.matmul(
                            ps[:, 0:DG],
                            lhsT=xn[:, k, sst * P : (sst + 1) * P],
                            rhs=wv_sb[:, k, :],
                            start=(k == 0),
                            stop=(k == KSUB - 1),
                        )
                    psv = ps[:, 0:DG].rearrange("p (h d) -> p h d", h=HC, d=HD)
                    nc.vector.tensor_copy(
                        out=v_sb[:, st0 + sst, 0:HC:2, 0:HD], in_=psv[:, 0:HC:2, :]
                    )
                    nc.vector.tensor_copy(
                        out=v_sb[:, st0 + sst, 1:HC:2, HD:P], in_=psv[:, 1:HC:2, :]
                    )
                return u

            units.append(qchunk(0))
            units.append(qchunk(1))
            units.append(kchunk(0))
            units.append(kchunk(1))
            for sst in range(SQT // P):
                units.append(vchunk(sst))
            return units

        def attn_units(n):
            """Attention steps for sq-tile n: per head pair hm, one unit per
            ski (scores even+odd packed, exp, mask, pipelined PV) plus two
            norm units."""
            nski = 4 * n + 4
            sq0 = n * SQT
            nsl = slice(sq0, sq0 + SQT)
            units = []
            def emit_pv(state, pend, hm, nski):
                ski, w0, pt = pend
                nc.tensor.matmul(
                    state["cpsA"][:, w0:],
                    lhsT=v_sb[:, ski, 2 * hm, :],
                    rhs=pt[:, w0:SQT],
                    start=(ski == 0),
                    stop=(ski == nski - 1),
                )
                nc.tensor.matmul(
                    state["cpsB"][:, w0:],
                    lhsT=v_sb[:, ski, 2 * hm + 1, :],
                    rhs=pt[:, SQT + w0 :],
                    start=(ski == 0),
                    stop=(ski == nski - 1),
                )

            for hm in range(DG // P):
                state = {"pend": None, "cpsA": None, "cpsB": None,
                         "recs": None, "scrs": None}

                def step(ski, hm=hm, state=state, n=n, nski=nski, sq0=sq0):
                    def u():
                        if ski == 0:
                            state["cpsA"] = cps.tile(
                                [P, SQT], F32, tag="ctx", name=f"cA_{n}_{hm}"
                            )
                            state["cpsB"] = cps.tile(
                                [P, SQT], F32, tag="ctx", name=f"cB_{n}_{hm}"
                            )
                        diag = ski >= 4 * n
                        w0 = (128 * ski - sq0) if diag else 0
                        spsum = sps.tile(
                            [P, 2 * SQT], F32, tag="s", name=f"s_{n}_{hm}_{ski}"
                        )
                        pt = ptp.tile(
                            [P, 2 * SQT], BF, tag="pt", name=f"pt_{n}_{hm}_{ski}"
                        )
                        # the two heads' scores; kt zero-padding makes the
                        # other head's q rows contract to 0
                        nc.tensor.matmul(
                            spsum[:, w0:SQT],
                            lhsT=kt_sb[:, 2 * hm, ski * P : (ski + 1) * P],
                            rhs=qt_sb[:, hm, sq0 + w0 : sq0 + SQT],
                            start=True,
                            stop=True,
                        )
                        nc.tensor.matmul(
                            spsum[:, SQT + w0 :],
                            lhsT=kt_sb[:, 2 * hm + 1, ski * P : (ski + 1) * P],
                            rhs=qt_sb[:, hm, sq0 + w0 : sq0 + SQT],
                            start=True,
                            stop=True,
                        )
                        if w0 >= 256:  # skip the big junk hole between halves
                            nc.scalar.activation(
                                out=pt[:, w0:SQT], in_=spsum[:, w0:SQT],
                                func=EXP, bias=0.0, scale=float(scale),
                            )
                            nc.scalar.activation(
                                out=pt[:, SQT + w0 :], in_=spsum[:, SQT + w0 :],
                                func=EXP, bias=0.0, scale=float(scale),
                            )
                        else:
                            nc.scalar.activation(
                                out=pt[:, w0:], in_=spsum[:, w0:],
                                func=EXP, bias=0.0, scale=float(scale),
                            )
                        if diag:  # zero entries with sk > sq in the diag block
                            for base in (0, SQT):
                                nc.gpsimd.affine_select(
                                    out=pt[:, base + w0 : base + w0 + P],
                                    in_=pt[:, base + w0 : base + w0 + P],
                                    pattern=[[1, P]],
                                    compare_op=mybir.AluOpType.is_ge,
                                    fill=0.0,
                                    base=0,
                                    channel_multiplier=-1,
                                )
                        if state["pend"] is not None:
                            emit_pv(state, state["pend"], hm, nski)
                        state["pend"] = (ski, w0, pt)
                    return u

                def normA(hm=hm, state=state, n=n, nski=nski):
                    def u():
                        emit_pv(state, state["pend"], hm, nski)
                        scrE = scr.tile([P, SQT], F32, tag="scr", name=f"scE_{n}_{hm}")
                        scrO = scr.tile([P, SQT], F32, tag="scr", name=f"scO_{n}_{hm}")
                        nc.vector.tensor_copy(out=scrE, in_=state["cpsA"])
                        nc.vector.tensor_copy(out=scrO, in_=state["cpsB"])
                        # scatter the denom rows across partitions: a [1, 512]
                        # DVE op runs on one lane (~3.3us); [128, 4] is ~26ns
                        sprE = smalls.tile([P, SQT // P], F32, tag="spr", name=f"spE_{n}_{hm}")
                        sprO = smalls.tile([P, SQT // P], F32, tag="spr", name=f"spO_{n}_{hm}")
                        nc.sync.dma_start(out=sprE, in_=scrE[64:65, :])
                        nc.sync.dma_start(out=sprO, in_=scrO[0:1, :])
                        state["scrs"] = (scrE, scrO)
                        state["sprs"] = (sprE, sprO)
                    return u

                def normA2(hm=hm, state=state, n=n):
                    def u():
                        sprE, sprO = state["sprs"]
                        rbE = smalls.tile([P, SQT // P], BF, tag="sprb", name=f"rbE_{n}_{hm}")
                        rbO = smalls.tile([P, SQT // P], BF, tag="sprb", name=f"rbO_{n}_{hm}")
                        with nc.allow_low_precision(reason="bf16 softmax denom"):
                            nc.vector.reciprocal(out=rbE, in_=sprE)
                            nc.vector.reciprocal(out=rbO, in_=sprO)
                        recE = smalls.tile([1, SQT], BF, tag="rec", name=f"rE_{n}_{hm}")
                        recO = smalls.tile([1, SQT], BF, tag="rec", name=f"rO_{n}_{hm}")
                        nc.sync.dma_start(out=recE, in_=rbE)
                        nc.sync.dma_start(out=recO, in_=rbO)
                        state["recs"] = (recE, recO)
                    return u

                def normB(hm=hm, state=state, n=n, nsl=nsl):
                    def u():
                        scrE, scrO = state["scrs"]
                        recE, recO = state["recs"]
                        bps = pps.tile([P, SQT], F32, tag="p", name=f"bc_{n}_{hm}")
                        nc.tensor.matmul(
                            bps, lhsT=ones_sb[0:1, 0, :], rhs=recE,
                            start=True, stop=False,
                        )
                        nc.tensor.matmul(
                            bps, lhsT=ones_sb[0:1, 1, :], rhs=recO,
                            start=False, stop=True,
                        )
                        nc.vector.tensor_tensor(
                            ctxt_sb[0:64, hm, nsl], scrE[0:64, :], bps[0:64, :],
                            mybir.AluOpType.mult,
                        )
                        nc.vector.tensor_tensor(
                            ctxt_sb[64:P, hm, nsl], scrO[64:P, :], bps[64:P, :],
                            mybir.AluOpType.mult,
                        )
                    return u

                for ski in range(nski):
                    units.append(step(ski))
                units.append(normA())
                units.append(normA2())
                nb = normB()
                units.append(lambda nb=nb: defer(nb, 6))
            return units

        def outproj_units(n, act_copies=False):
            """Partial output projection chunks for st tiles 4n..4n+3."""
            units = []
            ots = {}

            def chunk(st, nn):
                def u():
                    if nn == 0:
                        ots[st] = outp.tile([P, D], BF, tag="out", name=f"ot_{st}")
                    ot = ots[st]
                    ps = pps.tile([P, SQT], F32, tag="p", name=f"ops_{st}_{nn}")
                    for k in range(DG // P):
                        nc.tensor.matmul(
                            ps,
                            lhsT=ctxt_sb[:, k, st * P : (st + 1) * P],
                            rhs=wo_sb[:, k, nn * SQT : (nn + 1) * SQT],
                            start=(k == 0),
                            stop=(k == DG // P - 1),
                        )
                    if act_copies and (st + nn) % 2 == 0:
                        nc.scalar.activation(
                            out=ot[:, nn * SQT : (nn + 1) * SQT], in_=ps,
                            func=mybir.ActivationFunctionType.Copy,
                        )
                    else:
                        nc.vector.tensor_copy(
                            out=ot[:, nn * SQT : (nn + 1) * SQT], in_=ps
                        )
                    if nn == D // SQT - 1:
                        eng = (nc.scalar, nc.gpsimd)[st % 2]
                        eng.dma_start(out=out[st * P : (st + 1) * P, :], in_=ot)
                return u

            for st in range(4 * n, 4 * n + 4):
                for nn in range(D // SQT):
                    units.append(chunk(st, nn))
            return units

        deferred = []  # [(countdown, fn)] -- emitted a few units later so a
        # unit whose first instruction waits on a long non-PE chain (the norm
        # reciprocal's DMA scatter/gather) doesn't block the in-order PE queue

        def emit(u):
            u()
            for d in deferred:
                d[0] -= 1
            while deferred and deferred[0][0] <= 0:
                deferred.pop(0)[1]()

        def defer(fn, after):
            deferred.append([after, fn])

        def flush_deferred():
            while deferred:
                deferred.pop(0)[1]()

        def interleave(steps, fillers, reserve=0):
            """Emit steps with fillers spread evenly between them; the last
            `reserve` fillers are held back until after all steps (PE work to
            hide the final norm-chain latency)."""
            nf, ns = len(fillers) - reserve, len(steps)
            fi = 0
            for i, u in enumerate(steps):
                emit(u)
                want = (i + 1) * nf // ns
                while fi < want:
                    emit(fillers[fi])
                    fi += 1
            while fi < len(fillers):
                emit(fillers[fi])
                fi += 1

        # --- schedule ---
        emit_xdma(1)
        for u in proj_units(0):
            emit(u)
        emit_xdma(2)
        interleave(attn_units(0), proj_units(1))
        emit_xdma(3)
        interleave(attn_units(1), proj_units(2))
        nc.sync.dma_start(out=wo_sb, in_=wo)
        interleave(attn_units(2), proj_units(3))
        interleave(
            attn_units(3),
            outproj_units(0) + outproj_units(1) + outproj_units(2),
            reserve=4,
        )
        flush_deferred()
        for u in outproj_units(3, act_copies=True):
            emit(u)
        flush_deferred()


def build_nc():
    if "nc" in _CACHE:
        return _CACHE["nc"]
    nc = bacc.Bacc("TRN2", target_bir_lowering=False, debug=False, num_devices=NCORES)
    xT = nc.dram_tensor("xT", (D, S), BF, kind="ExternalInput").ap()
    wq = nc.dram_tensor("wq", (P, KSUB, DG), BF, kind="ExternalInput").ap()
    wk = nc.dram_tensor("wk", (P, KSUB, DG), BF, kind="ExternalInput").ap()
    wv = nc.dram_tensor("wv", (P, KSUB, DG), BF, kind="ExternalInput").ap()
    wo = nc.dram_tensor("wo", (P, DG // P, D), BF, kind="ExternalInput").ap()
    out = nc.dram_tensor("out", (S, D), BF, kind="ExternalOutput").ap()
    with tile.TileContext(nc) as tc:
        _mha_tile_kernel(tc, xT, wq, wk, wv, wo, out)
    nc.compile()
    _CACHE["nc"] = nc
    return nc


def make_in_maps(x, Wq, Wk, Wv, Wo):
    bf = ml_dtypes.bfloat16
    x = np.asarray(x, np.float32)
    in_maps = []
    for c in range(NCORES):
        b, g = c // GROUPS, c % GROUPS
        cols = slice(g * DG, (g + 1) * DG)

        def wslice(W):
            # [D, DG] -> [128, KSUB, DG] with [p, k, m] = W[k*128+p, m]
            return np.ascontiguousarray(
                np.asarray(W, np.float32)[:, cols]
                .reshape(KSUB, P, DG)
                .transpose(1, 0, 2)
                .astype(bf)
            )

        wo_c = np.ascontiguousarray(
            np.asarray(Wo, np.float32)[cols, :]
            .reshape(DG // P, P, D)
            .transpose(1, 0, 2)
            .astype(bf)
        )
        in_maps.append(
            {
                "xT": np.ascontiguousarray(x[b].T.astype(bf)),
                "wq": wslice(Wq),
                "wk": wslice(Wk),
                "wv": wslice(Wv),
                "wo": wo_c,
            }
        )
    return in_maps


def kernel(x, Wq, Wk, Wv, Wo, bo):
    nc = build_nc()
    in_maps = make_in_maps(x, Wq, Wk, Wv, Wo)
    trace = bool(int(os.environ.get("MHA_TRACE", "0")))
    if trace:
        _install_ntff_hook()
    res = run_bass_kernel_spmd(
        nc, in_maps, core_ids=list(range(NCORES)), trace=trace,
        trace_cores=list(range(NCORES)) if trace else None,
    )
    _CACHE["last_results"] = res
    bo = np.asarray(bo, np.float32)
    out = np.zeros((B, S, D), np.float32)
    for c in range(NCORES):
        out[c // GROUPS] += np.asarray(res.results[c]["out"], np.float32)
    out += bo[None, None, :]
    return out
